# revision 6
# baseline (speedup 1.0000x reference)
"""Multi-head causal self-attention (B=4, T=2048, D=1024, H=16) on 8 TRN2
NeuronCores.

Sharding: core c handles batch b = c//2 and half the heads (8 heads = 512
local dims).  Each core runs an identical Bass/Tile NEFF (SPMD, no
collectives) computing:

    Q^T = (s*Wq_slice) @ x_q^T          (512, 2048)  [spilled to DRAM]
    K^T = Wk_slice @ x_k^T              (512, 2048)  [SBUF resident]
    V   = x_v @ Wv_slice^T              (2048, 512)  [SBUF, +ones col/head]
    per (q-block, head):  S^T = K^T_chunk.T-matmuls, exp, P^T V via PE,
                          softmax denominator from an appended ones column
    out_partial = ctx @ Wo[:, slice].T  (2048, 1024)

The host sums the two partial outputs per batch (row-parallel output
projection) and adds the output bias.

Score scale 1/sqrt(64) is folded into Wq on the host.  bq/bk/bv are zero
for this problem's deterministic inputs; a numpy fallback covers the
general case.
"""

from contextlib import ExitStack

import numpy as np

import concourse.bass as bass
import concourse.tile as tile
from concourse import bass_utils, mybir
from concourse.tile_sem_assignment import N_PROCS
from concourse.vector_clock import ScopedClock, VectorClock

F32 = mybir.dt.float32

P = 128          # partition dim
T = 2048         # sequence length
DIN = 1024       # model dim
DLOC = 512       # local head dims per core (8 heads x 64)
NHL = 8          # local heads per core
DK = 64          # head dim
VSLOT = DK + 1   # V columns per head incl. the denominator ones column
NQ = 512         # q-block width (one fp32 PSUM bank)
KC = DIN // P    # 8  contraction chunks for projections
NT = T // NQ     # 4  t-blocks of 512
NTC = T // P     # 16 t-chunks of 128
NEG = -1.0e30
N_CORES = 8
EXP = mybir.ActivationFunctionType.Exp


class _SplitDrainTileContext(tile.TileContext):
    """Workaround: the walrus build in this container rejects a Drain
    instruction carrying more than a couple of sync waits ("Too many sync
    wait commands").  Emit one Drain per logical proc instead of the stock
    single Drain with one wait per proc."""

    def _drain_and_barrier(self, tick_clock, wait_clock):
        gc = tick_clock.global_clock
        for p in range(N_PROCS):
            if gc[p] > 0:
                sub = VectorClock([gc[q] if q == p else 0 for q in range(N_PROCS)])
                drain_inst = self.nc.sync.drain()
                wait_clock.add_sem_waits(drain_inst.ins, ScopedClock({None: sub}))
        self.nc.all_engine_barrier()
        assert self.sems is not None
        popped = self.nc._tile_sem_poison_stack.pop()
        assert popped is self._sem_poison
        self.nc.clear_and_free_semaphores(list(self.sems.allocated().values()))
        self.nc.all_engine_barrier()


_MAX_WAITS = 1  # this walrus build rejects instructions with more sync waits


def _split_excess_waits(nc: bass.Bass, max_waits: int = _MAX_WAITS) -> None:
    """Move sync waits beyond `max_waits` per instruction onto preceding
    single-wait EventSemaphore instructions on the same engine (same engine
    queue => executes first, so semantics are preserved)."""
    n = 0
    for f in nc.m.functions:
        for b in f.blocks:
            out = []
            changed = False
            for inst in b.instructions:
                si = inst.sync_info
                waits = list(si.on_wait) if si is not None and si.on_wait else []
                if len(waits) > max_waits:
                    for w in waits[:-max_waits]:
                        n += 1
                        out.append(
                            mybir.InstEventSemaphore(
                                name=f"xsplitw_{n}",
                                engine=inst.engine,
                                ins=[],
                                outs=[],
                                sync_info=mybir.SyncInfo(on_wait=[w], on_update=[]),
                            )
                        )
                    inst.sync_info = mybir.SyncInfo(
                        on_wait=waits[-max_waits:], on_update=list(si.on_update)
                    )
                    changed = True
                out.append(inst)
            if changed:
                b.instructions = out


def _build_program() -> bass.Bass:
    nc = bass.Bass(trn_type="TRN2", debug=False, num_devices=N_CORES)

    xq_d = nc.dram_tensor("xq", [DIN, T], F32, kind="ExternalInput").ap()
    xk_d = nc.dram_tensor("xk", [DIN, T], F32, kind="ExternalInput").ap()
    xv_d = nc.dram_tensor("xv", [DIN, T], F32, kind="ExternalInput").ap()
    wq_d = nc.dram_tensor("wq", [DIN, DLOC], F32, kind="ExternalInput").ap()
    wk_d = nc.dram_tensor("wk", [DIN, DLOC], F32, kind="ExternalInput").ap()
    wv_d = nc.dram_tensor("wv", [DIN, DLOC], F32, kind="ExternalInput").ap()
    wo_d = nc.dram_tensor("wo", [DLOC, DIN], F32, kind="ExternalInput").ap()
    mask_d = nc.dram_tensor("mask", [P, P], F32, kind="ExternalInput").ap()
    out_d = nc.dram_tensor("out", [T, DIN], F32, kind="ExternalOutput").ap()
    qt_d = nc.dram_tensor("qt_spill", [DLOC, T], F32).ap()

    with _SplitDrainTileContext(nc) as tc, ExitStack() as ctx:
        persist = ctx.enter_context(tc.tile_pool(name="persist", bufs=1))
        wpool = ctx.enter_context(tc.tile_pool(name="w", bufs=10))
        xpool = ctx.enter_context(tc.tile_pool(name="x", bufs=10))
        stage = ctx.enter_context(tc.tile_pool(name="stage", bufs=4))
        qrpool = ctx.enter_context(tc.tile_pool(name="qr", bufs=3))
        epool = ctx.enter_context(tc.tile_pool(name="e", bufs=4))
        rpool = ctx.enter_context(tc.tile_pool(name="r", bufs=2))
        rbpool = ctx.enter_context(tc.tile_pool(name="rb", bufs=2))
        ps_proj = ctx.enter_context(tc.tile_pool(name="ps_proj", bufs=2, space="PSUM"))
        ps_s = ctx.enter_context(tc.tile_pool(name="ps_s", bufs=2, space="PSUM"))
        ps_ctx = ctx.enter_context(tc.tile_pool(name="ps_ctx", bufs=2, space="PSUM"))
        ps_bc = ctx.enter_context(tc.tile_pool(name="ps_bc", bufs=1, space="PSUM"))

        # ---- persistent SBUF buffers ----
        kt = [persist.tile([P, T], F32, name=f"kt{i}", tag=f"kt{i}") for i in range(4)]
        ctxt = [
            persist.tile([P, T], F32, name=f"ctxt{i}", tag=f"ctxt{i}") for i in range(4)
        ]
        va = persist.tile([P, NTC * NHL * VSLOT], F32, name="va", tag="va")
        mask_sb = persist.tile([P, P], F32, name="mask_sb", tag="mask")
        ones_sb = persist.tile([1, DK], F32, name="ones_sb", tag="ones")

        nc.sync.dma_start(out=mask_sb, in_=mask_d)
        nc.vector.memset(ones_sb, 1.0)
        va_view = va.rearrange("p (t h e) -> p t h e", h=NHL, e=VSLOT)
        nc.vector.memset(va_view[:, :, :, DK : DK + 1], 1.0)

        # ---- V projection:  V[t, dv] = sum_k x_v^T[k, t] * Wv^T[k, dv] ----
        wv_sb = []
        for kc in range(KC):
            wt = wpool.tile([P, DLOC], F32, name=f"wv{kc}", tag="w")
            nc.sync.dma_start(out=wt, in_=wv_d[kc * P : (kc + 1) * P, :])
            wv_sb.append(wt)
        for tg in range(NT):
            xcs = []
            for kc in range(KC):
                xc = xpool.tile([P, NQ], F32, name=f"xv_{tg}_{kc}", tag="x")
                nc.sync.dma_start(
                    out=xc, in_=xv_d[kc * P : (kc + 1) * P, tg * NQ : (tg + 1) * NQ]
                )
                xcs.append(xc)
            for half in range(2):
                psums = [
                    ps_proj.tile([P, DLOC], F32, name=f"vps{half}_{i}", tag="pp")
                    for i in range(2)
                ]
                for kc in range(KC):
                    for i in range(2):
                        tsub = half * 2 + i
                        nc.tensor.matmul(
                            psums[i],
                            lhsT=xcs[kc][:, tsub * P : (tsub + 1) * P],
                            rhs=wv_sb[kc],
                            start=(kc == 0),
                            stop=(kc == KC - 1),
                        )
                for i in range(2):
                    tci = tg * 4 + half * 2 + i
                    nc.vector.tensor_copy(
                        out=va_view[:, tci, :, 0:DK],
                        in_=psums[i].rearrange("p (h e) -> p h e", e=DK),
                    )

        # ---- Q^T / K^T projections: out[m, t] = sum_k W^T[k, m] x^T[k, t] ----
        def qk_proj(w_dram, x_dram, sink, label):
            w_sb = []
            for kc in range(KC):
                wt = wpool.tile([P, DLOC], F32, name=f"w{label}{kc}", tag="w")
                nc.sync.dma_start(out=wt, in_=w_dram[kc * P : (kc + 1) * P, :])
                w_sb.append(wt)
            for nt in range(NT):
                xcs = []
                for kc in range(KC):
                    xc = xpool.tile([P, NQ], F32, name=f"x{label}_{nt}_{kc}", tag="x")
                    nc.sync.dma_start(
                        out=xc, in_=x_dram[kc * P : (kc + 1) * P, nt * NQ : (nt + 1) * NQ]
                    )
                    xcs.append(xc)
                for mh in range(2):
                    psums = [
                        ps_proj.tile([P, NQ], F32, name=f"{label}ps{mh}_{i}", tag="pp")
                        for i in range(2)
                    ]
                    for kc in range(KC):
                        for i in range(2):
                            mq = mh * 2 + i
                            nc.tensor.matmul(
                                psums[i],
                                lhsT=w_sb[kc][:, mq * P : (mq + 1) * P],
                                rhs=xcs[kc],
                                start=(kc == 0),
                                stop=(kc == KC - 1),
                            )
                    for i in range(2):
                        sink(mh * 2 + i, nt, psums[i])

        def q_sink(mq, nt, psum):
            st = stage.tile([P, NQ], F32, name=f"qst{mq}_{nt}", tag="stage")
            nc.vector.tensor_copy(out=st, in_=psum)
            nc.sync.dma_start(
                out=qt_d[mq * P : (mq + 1) * P, nt * NQ : (nt + 1) * NQ], in_=st
            )

        def k_sink(mq, nt, psum):
            nc.vector.tensor_copy(
                out=kt[mq][:, nt * NQ : (nt + 1) * NQ], in_=psum
            )

        qk_proj(wq_d, xq_d, q_sink, "q")
        qk_proj(wk_d, xk_d, k_sink, "k")

        # ---- attention: per q-block, per head ----
        for qi in range(NT):
            jmax = 4 * (qi + 1)
            for hp in range(NHL // 2):
                # two heads per 128-partition Q^T tile, matching the K^T
                # layout so lhsT/rhs share a base partition
                qt_t = qrpool.tile([P, NQ], F32, name=f"qt{qi}_{hp}", tag="qr")
                nc.sync.dma_start(
                    out=qt_t,
                    in_=qt_d[hp * P : (hp + 1) * P, qi * NQ : (qi + 1) * NQ],
                )
                for sub in range(2):
                    h = 2 * hp + sub
                    ktile = kt[h // 2]
                    krow = (h % 2) * DK
                    cps = ps_ctx.tile([VSLOT, NQ], F32, name=f"cps{qi}_{h}", tag="ctx")

                    # software-pipelined: ctx matmul of step j-1 issues after
                    # the scores matmul of step j, keeping PE fed while
                    # exp(j-1) runs
                    pend = None  # (et, off, j)
                    for j in range(jmax):
                        delta = j * P - qi * NQ
                        off = max(0, delta)
                        sps = ps_s.tile(
                            [P, NQ], F32, name=f"sps{qi}_{h}_{j}", tag="s"
                        )
                        nc.tensor.matmul(
                            sps[:, off:NQ],
                            lhsT=ktile[krow : krow + DK, j * P : (j + 1) * P],
                            rhs=qt_t[krow : krow + DK, off:NQ],
                            start=True,
                            stop=True,
                        )
                        if pend is not None:
                            pet, poff, pj = pend
                            nc.tensor.matmul(
                                cps if pj == 0 else cps[:, poff:NQ],
                                lhsT=va_view[:, pj, h, :],
                                rhs=pet if pj == 0 else pet[:, poff:NQ],
                                start=(pj == 0),
                                stop=False,
                                skip_group_check=True,
                            )
                        if delta >= 0:
                            nc.vector.tensor_add(
                                sps[:, off : off + P], sps[:, off : off + P], mask_sb
                            )
                        et = epool.tile([P, NQ], F32, name=f"et{qi}_{h}_{j}", tag="e")
                        nc.scalar.activation(
                            out=et[:, off:NQ], in_=sps[:, off:NQ], func=EXP
                        )
                        pend = (et, off, j)
                    pet, poff, pj = pend
                    nc.tensor.matmul(
                        cps if pj == 0 else cps[:, poff:NQ],
                        lhsT=va_view[:, pj, h, :],
                        rhs=pet if pj == 0 else pet[:, poff:NQ],
                        start=(pj == 0),
                        stop=True,
                        skip_group_check=True,
                    )

                    # normalize: ctx[dv, q] / denom[q]
                    rt = rpool.tile([1, NQ], F32, name=f"rt{qi}_{h}", tag="recip")
                    nc.vector.reciprocal(rt, cps[DK : DK + 1, :])
                    bc = ps_bc.tile([DK, NQ], F32, name=f"bc{qi}_{h}", tag="bc")
                    nc.tensor.matmul(bc, lhsT=ones_sb, rhs=rt, start=True, stop=True)
                    rb = rbpool.tile([DK, NQ], F32, name=f"rb{qi}_{h}", tag="rb")
                    nc.vector.tensor_copy(out=rb, in_=bc)
                    nc.vector.tensor_mul(
                        ctxt[h // 2][krow : krow + DK, qi * NQ : (qi + 1) * NQ],
                        cps[0:DK, :],
                        rb,
                    )

        # ---- output projection: out[t, n] = sum_dl ctx^T[dl, t] Wo^T[dl, n] ----
        wo_sb = []
        for kc4 in range(4):
            row = []
            for n in range(2):
                wt = wpool.tile([P, NQ], F32, name=f"wo{kc4}_{n}", tag="w")
                nc.sync.dma_start(
                    out=wt,
                    in_=wo_d[kc4 * P : (kc4 + 1) * P, n * NQ : (n + 1) * NQ],
                )
                row.append(wt)
            wo_sb.append(row)
        for tci in range(NTC):
            for n in range(2):
                ops = ps_proj.tile([P, NQ], F32, name=f"ops{tci}_{n}", tag="pp")
                for kc4 in range(4):
                    nc.tensor.matmul(
                        ops,
                        lhsT=ctxt[kc4][:, tci * P : (tci + 1) * P],
                        rhs=wo_sb[kc4][n],
                        start=(kc4 == 0),
                        stop=(kc4 == 3),
                    )
                st = stage.tile([P, NQ], F32, name=f"ost{tci}_{n}", tag="stage")
                nc.vector.tensor_copy(out=st, in_=ops)
                nc.sync.dma_start(
                    out=out_d[tci * P : (tci + 1) * P, n * NQ : (n + 1) * NQ], in_=st
                )

    _split_excess_waits(nc)
    return nc


_NC_CACHE: bass.Bass | None = None


def _get_program() -> bass.Bass:
    global _NC_CACHE
    if _NC_CACHE is None:
        _NC_CACHE = _build_program()
    return _NC_CACHE


def _numpy_reference(q, k, v, Wq, Wk, Wv, Wo, bq, bk, bv, bo):
    """Exact fallback, used only if bq/bk/bv are nonzero (never the case for
    this problem's deterministic inputs)."""
    B, T_, D = q.shape
    H = 16
    dk = D // H

    def split(x):
        return x.reshape(B, T_, H, dk).transpose(0, 2, 1, 3)

    qh = split(q @ Wq.T + bq)
    kh = split(k @ Wk.T + bk)
    vh = split(v @ Wv.T + bv)
    scores = np.einsum("bhqd,bhkd->bhqk", qh, kh) / np.sqrt(np.float32(dk))
    causal = np.tril(np.ones((T_, T_), dtype=bool))
    scores = np.where(causal, scores, -np.inf).astype(np.float32)
    scores -= scores.max(axis=-1, keepdims=True)
    e = np.exp(scores)
    attn = e / e.sum(axis=-1, keepdims=True)
    ctx = np.einsum("bhqk,bhkd->bhqd", attn, vh)
    merged = ctx.transpose(0, 2, 1, 3).reshape(B, T_, D)
    return (merged @ Wo.T + bo).astype(np.float32)


def kernel(q, k, v, Wq, Wk, Wv, Wo, bq, bk, bv, bo):
    q, k, v = (np.asarray(a, np.float32) for a in (q, k, v))
    Wq, Wk, Wv, Wo = (np.asarray(a, np.float32) for a in (Wq, Wk, Wv, Wo))
    bq, bk, bv, bo = (np.asarray(a, np.float32) for a in (bq, bk, bv, bo))

    if np.any(bq) or np.any(bk) or np.any(bv):
        return _numpy_reference(q, k, v, Wq, Wk, Wv, Wo, bq, bk, bv, bo)

    B = q.shape[0]
    scale = np.float32(1.0 / np.sqrt(DK))
    wq_s = (Wq * scale).T  # fold score scale into Wq
    wk_s = Wk.T
    wv_s = Wv.T
    mask = np.where(
        np.arange(P)[:, None] <= np.arange(P)[None, :], 0.0, NEG
    ).astype(np.float32)

    in_maps = []
    for c in range(N_CORES):
        b, hh = divmod(c, 2)
        hs = slice(hh * DLOC, (hh + 1) * DLOC)
        in_maps.append(
            {
                "xq": np.ascontiguousarray(q[b].T),
                "xk": np.ascontiguousarray(k[b].T),
                "xv": np.ascontiguousarray(v[b].T),
                "wq": np.ascontiguousarray(wq_s[:, hs]),
                "wk": np.ascontiguousarray(wk_s[:, hs]),
                "wv": np.ascontiguousarray(wv_s[:, hs]),
                "wo": np.ascontiguousarray(Wo[:, hs].T),
                "mask": mask,
            }
        )

    nc = _get_program()
    res = bass_utils.run_bass_kernel_spmd(nc, in_maps, core_ids=list(range(N_CORES)))

    out = np.empty((B, T, DIN), np.float32)
    for b in range(B):
        out[b] = res.results[2 * b]["out"] + res.results[2 * b + 1]["out"]
    out += bo
    return out


# revision 8
# speedup vs baseline: 2.5155x; 2.5155x over previous
"""Multi-head causal self-attention (B=4, T=2048, D=1024, H=16) on 8 TRN2
NeuronCores.

Sharding: core c handles batch b = c//2 and half the heads (8 heads = 512
local dims).  Each core runs an identical Bass/Tile NEFF (SPMD, no
collectives) computing:

    Q^T = (s*Wq_slice) @ x_q^T          (512, 2048)  [spilled to DRAM]
    K^T = Wk_slice @ x_k^T              (512, 2048)  [SBUF resident]
    V   = x_v @ Wv_slice^T              (2048, 512)  [SBUF, +ones col/head]
    per (q-block, head):  S^T = K^T_chunk.T-matmuls, exp, P^T V via PE,
                          softmax denominator from an appended ones column
    out_partial = ctx @ Wo[:, slice].T  (2048, 1024)

The host sums the two partial outputs per batch (row-parallel output
projection) and adds the output bias.

Score scale 1/sqrt(64) is folded into Wq on the host.  bq/bk/bv are zero
for this problem's deterministic inputs; a numpy fallback covers the
general case.
"""

from contextlib import ExitStack

import numpy as np

import concourse.bass as bass
import concourse.tile as tile
from concourse import bass_utils, mybir
from concourse.tile_sem_assignment import N_PROCS
from concourse.vector_clock import ScopedClock, VectorClock

F32 = mybir.dt.float32
F32R = mybir.dt.float32r

P = 128          # partition dim
T = 2048         # sequence length
DIN = 1024       # model dim
DLOC = 512       # local head dims per core (8 heads x 64)
NHL = 8          # local heads per core
DK = 64          # head dim
VSLOT = DK + 1   # V columns per head incl. the denominator ones column
NQ = 512         # q-block width (one fp32 PSUM bank)
KC = DIN // P    # 8  contraction chunks for projections
NT = T // NQ     # 4  t-blocks of 512
NTC = T // P     # 16 t-chunks of 128
NEG = -1.0e30
N_CORES = 8
EXP = mybir.ActivationFunctionType.Exp


class _SplitDrainTileContext(tile.TileContext):
    """Workaround: the walrus build in this container rejects a Drain
    instruction carrying more than a couple of sync waits ("Too many sync
    wait commands").  Emit one Drain per logical proc instead of the stock
    single Drain with one wait per proc."""

    def _drain_and_barrier(self, tick_clock, wait_clock):
        gc = tick_clock.global_clock
        for p in range(N_PROCS):
            if gc[p] > 0:
                sub = VectorClock([gc[q] if q == p else 0 for q in range(N_PROCS)])
                drain_inst = self.nc.sync.drain()
                wait_clock.add_sem_waits(drain_inst.ins, ScopedClock({None: sub}))
        self.nc.all_engine_barrier()
        assert self.sems is not None
        popped = self.nc._tile_sem_poison_stack.pop()
        assert popped is self._sem_poison
        self.nc.clear_and_free_semaphores(list(self.sems.allocated().values()))
        self.nc.all_engine_barrier()


_MAX_WAITS = 1  # this walrus build rejects instructions with more sync waits


def _split_excess_waits(nc: bass.Bass, max_waits: int = _MAX_WAITS) -> None:
    """Move sync waits beyond `max_waits` per instruction onto preceding
    single-wait EventSemaphore instructions on the same engine (same engine
    queue => executes first, so semantics are preserved)."""
    n = 0
    for f in nc.m.functions:
        for b in f.blocks:
            out = []
            changed = False
            for inst in b.instructions:
                si = inst.sync_info
                waits = list(si.on_wait) if si is not None and si.on_wait else []
                if len(waits) > max_waits:
                    for w in waits[:-max_waits]:
                        n += 1
                        out.append(
                            mybir.InstEventSemaphore(
                                name=f"xsplitw_{n}",
                                engine=inst.engine,
                                ins=[],
                                outs=[],
                                sync_info=mybir.SyncInfo(on_wait=[w], on_update=[]),
                            )
                        )
                    inst.sync_info = mybir.SyncInfo(
                        on_wait=waits[-max_waits:], on_update=list(si.on_update)
                    )
                    changed = True
                out.append(inst)
            if changed:
                b.instructions = out


def _build_program() -> bass.Bass:
    nc = bass.Bass(trn_type="TRN2", debug=False, num_devices=N_CORES)

    xq_d = nc.dram_tensor("xq", [DIN, T], F32R, kind="ExternalInput").ap()
    xk_d = nc.dram_tensor("xk", [DIN, T], F32R, kind="ExternalInput").ap()
    xv_d = nc.dram_tensor("xv", [DIN, T], F32R, kind="ExternalInput").ap()
    wq_d = nc.dram_tensor("wq", [DIN, DLOC], F32R, kind="ExternalInput").ap()
    wk_d = nc.dram_tensor("wk", [DIN, DLOC], F32R, kind="ExternalInput").ap()
    wv_d = nc.dram_tensor("wv", [DIN, DLOC], F32R, kind="ExternalInput").ap()
    wo_d = nc.dram_tensor("wo", [DLOC, DIN], F32R, kind="ExternalInput").ap()
    mask_d = nc.dram_tensor("mask", [P, P], F32, kind="ExternalInput").ap()
    out_d = nc.dram_tensor("out", [T, DIN], F32, kind="ExternalOutput").ap()
    qt_d = nc.dram_tensor("qt_spill", [DLOC, T], F32R).ap()

    with nc.allow_low_precision(
        reason="fp32r matmuls: 4x PE throughput, ~2e-4 rel err"
    ), _SplitDrainTileContext(nc) as tc, ExitStack() as ctx:
        persist = ctx.enter_context(tc.tile_pool(name="persist", bufs=1))
        wpool = ctx.enter_context(tc.tile_pool(name="w", bufs=10))
        xpool = ctx.enter_context(tc.tile_pool(name="x", bufs=10))
        stage = ctx.enter_context(tc.tile_pool(name="stage", bufs=4))
        qrpool = ctx.enter_context(tc.tile_pool(name="qr", bufs=3))
        epool = ctx.enter_context(tc.tile_pool(name="e", bufs=4))
        rpool = ctx.enter_context(tc.tile_pool(name="r", bufs=2))
        rbpool = ctx.enter_context(tc.tile_pool(name="rb", bufs=2))
        ps_proj = ctx.enter_context(tc.tile_pool(name="ps_proj", bufs=2, space="PSUM"))
        ps_s = ctx.enter_context(tc.tile_pool(name="ps_s", bufs=2, space="PSUM"))
        ps_ctx = ctx.enter_context(tc.tile_pool(name="ps_ctx", bufs=2, space="PSUM"))
        ps_bc = ctx.enter_context(tc.tile_pool(name="ps_bc", bufs=1, space="PSUM"))

        # ---- persistent SBUF buffers ----
        kt = [persist.tile([P, T], F32R, name=f"kt{i}", tag=f"kt{i}") for i in range(4)]
        ctxt = [
            persist.tile([P, T], F32R, name=f"ctxt{i}", tag=f"ctxt{i}") for i in range(4)
        ]
        va = persist.tile([P, NTC * NHL * VSLOT], F32R, name="va", tag="va")
        mask_sb = persist.tile([P, P], F32, name="mask_sb", tag="mask")
        ones_sb = persist.tile([1, DK], F32R, name="ones_sb", tag="ones")

        nc.sync.dma_start(out=mask_sb, in_=mask_d)
        # memset through f32 bitcast views: f32r memset fails an ISA check in
        # this walrus build, and 1.0 has identical bits in both formats
        nc.vector.memset(ones_sb.bitcast(F32), 1.0)
        va_view = va.rearrange("p (t h e) -> p t h e", h=NHL, e=VSLOT)
        va_view_f32 = va.bitcast(F32).rearrange("p (t h e) -> p t h e", h=NHL, e=VSLOT)
        nc.vector.memset(va_view_f32[:, :, :, DK : DK + 1], 1.0)

        # ---- V projection:  V[t, dv] = sum_k x_v^T[k, t] * Wv^T[k, dv] ----
        wv_sb = []
        for kc in range(KC):
            wt = wpool.tile([P, DLOC], F32R, name=f"wv{kc}", tag="w")
            nc.sync.dma_start(out=wt, in_=wv_d[kc * P : (kc + 1) * P, :])
            wv_sb.append(wt)
        for tg in range(NT):
            xcs = []
            for kc in range(KC):
                xc = xpool.tile([P, NQ], F32R, name=f"xv_{tg}_{kc}", tag="x")
                nc.sync.dma_start(
                    out=xc, in_=xv_d[kc * P : (kc + 1) * P, tg * NQ : (tg + 1) * NQ]
                )
                xcs.append(xc)
            for half in range(2):
                psums = [
                    ps_proj.tile([P, DLOC], F32, name=f"vps{half}_{i}", tag="pp")
                    for i in range(2)
                ]
                for kc in range(KC):
                    for i in range(2):
                        tsub = half * 2 + i
                        nc.tensor.matmul(
                            psums[i],
                            lhsT=xcs[kc][:, tsub * P : (tsub + 1) * P],
                            rhs=wv_sb[kc],
                            start=(kc == 0),
                            stop=(kc == KC - 1),
                        )
                for i in range(2):
                    tci = tg * 4 + half * 2 + i
                    nc.vector.tensor_copy(
                        out=va_view[:, tci, :, 0:DK],
                        in_=psums[i].rearrange("p (h e) -> p h e", e=DK),
                    )

        # ---- Q^T / K^T projections: out[m, t] = sum_k W^T[k, m] x^T[k, t] ----
        def qk_proj(w_dram, x_dram, sink, label):
            w_sb = []
            for kc in range(KC):
                wt = wpool.tile([P, DLOC], F32R, name=f"w{label}{kc}", tag="w")
                nc.sync.dma_start(out=wt, in_=w_dram[kc * P : (kc + 1) * P, :])
                w_sb.append(wt)
            for nt in range(NT):
                xcs = []
                for kc in range(KC):
                    xc = xpool.tile([P, NQ], F32R, name=f"x{label}_{nt}_{kc}", tag="x")
                    nc.sync.dma_start(
                        out=xc, in_=x_dram[kc * P : (kc + 1) * P, nt * NQ : (nt + 1) * NQ]
                    )
                    xcs.append(xc)
                for mh in range(2):
                    psums = [
                        ps_proj.tile([P, NQ], F32, name=f"{label}ps{mh}_{i}", tag="pp")
                        for i in range(2)
                    ]
                    for kc in range(KC):
                        for i in range(2):
                            mq = mh * 2 + i
                            nc.tensor.matmul(
                                psums[i],
                                lhsT=w_sb[kc][:, mq * P : (mq + 1) * P],
                                rhs=xcs[kc],
                                start=(kc == 0),
                                stop=(kc == KC - 1),
                            )
                    for i in range(2):
                        sink(mh * 2 + i, nt, psums[i])

        def q_sink(mq, nt, psum):
            st = stage.tile([P, NQ], F32R, name=f"qst{mq}_{nt}", tag="stage")
            nc.vector.tensor_copy(out=st, in_=psum)
            nc.sync.dma_start(
                out=qt_d[mq * P : (mq + 1) * P, nt * NQ : (nt + 1) * NQ], in_=st
            )

        def k_sink(mq, nt, psum):
            nc.vector.tensor_copy(
                out=kt[mq][:, nt * NQ : (nt + 1) * NQ], in_=psum
            )

        qk_proj(wq_d, xq_d, q_sink, "q")
        qk_proj(wk_d, xk_d, k_sink, "k")

        # ---- attention: per q-block, per head ----
        for qi in range(NT):
            jmax = 4 * (qi + 1)
            for hp in range(NHL // 2):
                # two heads per 128-partition Q^T tile, matching the K^T
                # layout so lhsT/rhs share a base partition
                qt_t = qrpool.tile([P, NQ], F32R, name=f"qt{qi}_{hp}", tag="qr")
                nc.sync.dma_start(
                    out=qt_t,
                    in_=qt_d[hp * P : (hp + 1) * P, qi * NQ : (qi + 1) * NQ],
                )
                for sub in range(2):
                    h = 2 * hp + sub
                    ktile = kt[h // 2]
                    krow = (h % 2) * DK
                    cps = ps_ctx.tile([VSLOT, NQ], F32, name=f"cps{qi}_{h}", tag="ctx")

                    # software-pipelined: ctx matmul of step j-1 issues after
                    # the scores matmul of step j, keeping PE fed while
                    # exp(j-1) runs
                    pend = None  # (et, off, j)
                    for j in range(jmax):
                        delta = j * P - qi * NQ
                        off = max(0, delta)
                        sps = ps_s.tile(
                            [P, NQ], F32, name=f"sps{qi}_{h}_{j}", tag="s"
                        )
                        nc.tensor.matmul(
                            sps[:, off:NQ],
                            lhsT=ktile[krow : krow + DK, j * P : (j + 1) * P],
                            rhs=qt_t[krow : krow + DK, off:NQ],
                            start=True,
                            stop=True,
                        )
                        if pend is not None:
                            pet, poff, pj = pend
                            nc.tensor.matmul(
                                cps if pj == 0 else cps[:, poff:NQ],
                                lhsT=va_view[:, pj, h, :],
                                rhs=pet if pj == 0 else pet[:, poff:NQ],
                                start=(pj == 0),
                                stop=False,
                                skip_group_check=True,
                            )
                        if delta >= 0:
                            nc.vector.tensor_add(
                                sps[:, off : off + P], sps[:, off : off + P], mask_sb
                            )
                        et = epool.tile([P, NQ], F32R, name=f"et{qi}_{h}_{j}", tag="e")
                        nc.scalar.activation(
                            out=et[:, off:NQ], in_=sps[:, off:NQ], func=EXP
                        )
                        pend = (et, off, j)
                    pet, poff, pj = pend
                    nc.tensor.matmul(
                        cps if pj == 0 else cps[:, poff:NQ],
                        lhsT=va_view[:, pj, h, :],
                        rhs=pet if pj == 0 else pet[:, poff:NQ],
                        start=(pj == 0),
                        stop=True,
                        skip_group_check=True,
                    )

                    # normalize: ctx[dv, q] / denom[q]
                    rt = rpool.tile([1, NQ], F32R, name=f"rt{qi}_{h}", tag="recip")
                    nc.vector.reciprocal(rt, cps[DK : DK + 1, :])
                    bc = ps_bc.tile([DK, NQ], F32, name=f"bc{qi}_{h}", tag="bc")
                    nc.tensor.matmul(bc, lhsT=ones_sb, rhs=rt, start=True, stop=True)
                    rb = rbpool.tile([DK, NQ], F32, name=f"rb{qi}_{h}", tag="rb")
                    nc.vector.tensor_copy(out=rb, in_=bc)
                    nc.vector.tensor_mul(
                        ctxt[h // 2][krow : krow + DK, qi * NQ : (qi + 1) * NQ],
                        cps[0:DK, :],
                        rb,
                    )

        # ---- output projection: out[t, n] = sum_dl ctx^T[dl, t] Wo^T[dl, n] ----
        wo_sb = []
        for kc4 in range(4):
            row = []
            for n in range(2):
                wt = wpool.tile([P, NQ], F32R, name=f"wo{kc4}_{n}", tag="w")
                nc.sync.dma_start(
                    out=wt,
                    in_=wo_d[kc4 * P : (kc4 + 1) * P, n * NQ : (n + 1) * NQ],
                )
                row.append(wt)
            wo_sb.append(row)
        for tci in range(NTC):
            for n in range(2):
                ops = ps_proj.tile([P, NQ], F32, name=f"ops{tci}_{n}", tag="pp")
                for kc4 in range(4):
                    nc.tensor.matmul(
                        ops,
                        lhsT=ctxt[kc4][:, tci * P : (tci + 1) * P],
                        rhs=wo_sb[kc4][n],
                        start=(kc4 == 0),
                        stop=(kc4 == 3),
                    )
                st = stage.tile([P, NQ], F32, name=f"ost{tci}_{n}", tag="stage")
                nc.vector.tensor_copy(out=st, in_=ops)
                nc.sync.dma_start(
                    out=out_d[tci * P : (tci + 1) * P, n * NQ : (n + 1) * NQ], in_=st
                )

    _split_excess_waits(nc)
    return nc


_NC_CACHE: bass.Bass | None = None


def _get_program() -> bass.Bass:
    global _NC_CACHE
    if _NC_CACHE is None:
        _NC_CACHE = _build_program()
    return _NC_CACHE


def _numpy_reference(q, k, v, Wq, Wk, Wv, Wo, bq, bk, bv, bo):
    """Exact fallback, used only if bq/bk/bv are nonzero (never the case for
    this problem's deterministic inputs)."""
    B, T_, D = q.shape
    H = 16
    dk = D // H

    def split(x):
        return x.reshape(B, T_, H, dk).transpose(0, 2, 1, 3)

    qh = split(q @ Wq.T + bq)
    kh = split(k @ Wk.T + bk)
    vh = split(v @ Wv.T + bv)
    scores = np.einsum("bhqd,bhkd->bhqk", qh, kh) / np.sqrt(np.float32(dk))
    causal = np.tril(np.ones((T_, T_), dtype=bool))
    scores = np.where(causal, scores, -np.inf).astype(np.float32)
    scores -= scores.max(axis=-1, keepdims=True)
    e = np.exp(scores)
    attn = e / e.sum(axis=-1, keepdims=True)
    ctx = np.einsum("bhqk,bhkd->bhqd", attn, vh)
    merged = ctx.transpose(0, 2, 1, 3).reshape(B, T_, D)
    return (merged @ Wo.T + bo).astype(np.float32)


def kernel(q, k, v, Wq, Wk, Wv, Wo, bq, bk, bv, bo):
    q, k, v = (np.asarray(a, np.float32) for a in (q, k, v))
    Wq, Wk, Wv, Wo = (np.asarray(a, np.float32) for a in (Wq, Wk, Wv, Wo))
    bq, bk, bv, bo = (np.asarray(a, np.float32) for a in (bq, bk, bv, bo))

    if np.any(bq) or np.any(bk) or np.any(bv):
        return _numpy_reference(q, k, v, Wq, Wk, Wv, Wo, bq, bk, bv, bo)

    B = q.shape[0]
    scale = np.float32(1.0 / np.sqrt(DK))
    wq_s = (Wq * scale).T  # fold score scale into Wq
    wk_s = Wk.T
    wv_s = Wv.T
    mask = np.where(
        np.arange(P)[:, None] <= np.arange(P)[None, :], 0.0, NEG
    ).astype(np.float32)

    in_maps = []
    for c in range(N_CORES):
        b, hh = divmod(c, 2)
        hs = slice(hh * DLOC, (hh + 1) * DLOC)
        in_maps.append(
            {
                "xq": np.ascontiguousarray(q[b].T),
                "xk": np.ascontiguousarray(k[b].T),
                "xv": np.ascontiguousarray(v[b].T),
                "wq": np.ascontiguousarray(wq_s[:, hs]),
                "wk": np.ascontiguousarray(wk_s[:, hs]),
                "wv": np.ascontiguousarray(wv_s[:, hs]),
                "wo": np.ascontiguousarray(Wo[:, hs].T),
                "mask": mask,
            }
        )

    nc = _get_program()
    res = bass_utils.run_bass_kernel_spmd(nc, in_maps, core_ids=list(range(N_CORES)))

    out = np.empty((B, T, DIN), np.float32)
    for b in range(B):
        out[b] = res.results[2 * b]["out"] + res.results[2 * b + 1]["out"]
    out += bo
    return out


# revision 10
# speedup vs baseline: 2.6713x; 1.0620x over previous
"""Multi-head causal self-attention (B=4, T=2048, D=1024, H=16) on 8 TRN2
NeuronCores.

Sharding: core c handles batch b = c//2 and half the heads (8 heads = 512
local dims).  Each core runs an identical Bass/Tile NEFF (SPMD, no
collectives) computing:

    Q^T = (s*Wq_slice) @ x_q^T          (512, 2048)  [spilled to DRAM]
    K^T = Wk_slice @ x_k^T              (512, 2048)  [SBUF resident]
    V   = x_v @ Wv_slice^T              (2048, 512)  [SBUF, +ones col/head]
    per (q-block, head):  S^T = K^T_chunk.T-matmuls, exp, P^T V via PE,
                          softmax denominator from an appended ones column
    out_partial = ctx @ Wo[:, slice].T  (2048, 1024)

The host sums the two partial outputs per batch (row-parallel output
projection) and adds the output bias.

Score scale 1/sqrt(64) is folded into Wq on the host.  bq/bk/bv are zero
for this problem's deterministic inputs; a numpy fallback covers the
general case.
"""

from contextlib import ExitStack

import numpy as np

import concourse.bass as bass
import concourse.tile as tile
from concourse import bass_utils, mybir
from concourse.tile_sem_assignment import N_PROCS
from concourse.vector_clock import ScopedClock, VectorClock

F32 = mybir.dt.float32
F32R = mybir.dt.float32r

P = 128          # partition dim
T = 2048         # sequence length
DIN = 1024       # model dim
DLOC = 512       # local head dims per core (8 heads x 64)
NHL = 8          # local heads per core
DK = 64          # head dim
VSLOT = DK + 1   # V columns per head incl. the denominator ones column
NQ = 512         # q-block width (one fp32 PSUM bank)
KC = DIN // P    # 8  contraction chunks for projections
NT = T // NQ     # 4  t-blocks of 512
NTC = T // P     # 16 t-chunks of 128
NEG = -1.0e30
N_CORES = 8
EXP = mybir.ActivationFunctionType.Exp


class _SplitDrainTileContext(tile.TileContext):
    """Workaround: the walrus build in this container rejects a Drain
    instruction carrying more than a couple of sync waits ("Too many sync
    wait commands").  Emit one Drain per logical proc instead of the stock
    single Drain with one wait per proc."""

    def _drain_and_barrier(self, tick_clock, wait_clock):
        gc = tick_clock.global_clock
        for p in range(N_PROCS):
            if gc[p] > 0:
                sub = VectorClock([gc[q] if q == p else 0 for q in range(N_PROCS)])
                drain_inst = self.nc.sync.drain()
                wait_clock.add_sem_waits(drain_inst.ins, ScopedClock({None: sub}))
        self.nc.all_engine_barrier()
        assert self.sems is not None
        popped = self.nc._tile_sem_poison_stack.pop()
        assert popped is self._sem_poison
        self.nc.clear_and_free_semaphores(list(self.sems.allocated().values()))
        self.nc.all_engine_barrier()


_MAX_WAITS = 1  # this walrus build rejects instructions with more sync waits


def _split_excess_waits(nc: bass.Bass, max_waits: int = _MAX_WAITS) -> None:
    """Move sync waits beyond `max_waits` per instruction onto preceding
    single-wait EventSemaphore instructions on the same engine (same engine
    queue => executes first, so semantics are preserved)."""
    n = 0
    for f in nc.m.functions:
        for b in f.blocks:
            out = []
            changed = False
            for inst in b.instructions:
                si = inst.sync_info
                waits = list(si.on_wait) if si is not None and si.on_wait else []
                if len(waits) > max_waits:
                    for w in waits[:-max_waits]:
                        n += 1
                        out.append(
                            mybir.InstEventSemaphore(
                                name=f"xsplitw_{n}",
                                engine=inst.engine,
                                ins=[],
                                outs=[],
                                sync_info=mybir.SyncInfo(on_wait=[w], on_update=[]),
                            )
                        )
                    inst.sync_info = mybir.SyncInfo(
                        on_wait=waits[-max_waits:], on_update=list(si.on_update)
                    )
                    changed = True
                out.append(inst)
            if changed:
                b.instructions = out


def _build_program() -> bass.Bass:
    nc = bass.Bass(trn_type="TRN2", debug=False, num_devices=N_CORES)

    xq_d = nc.dram_tensor("xq", [DIN, T], F32R, kind="ExternalInput").ap()
    xk_d = nc.dram_tensor("xk", [DIN, T], F32R, kind="ExternalInput").ap()
    xv_d = nc.dram_tensor("xv", [DIN, T], F32R, kind="ExternalInput").ap()
    wq_d = nc.dram_tensor("wq", [DIN, DLOC], F32R, kind="ExternalInput").ap()
    wk_d = nc.dram_tensor("wk", [DIN, DLOC], F32R, kind="ExternalInput").ap()
    wv_d = nc.dram_tensor("wv", [DIN, DLOC], F32R, kind="ExternalInput").ap()
    wo_d = nc.dram_tensor("wo", [DLOC, DIN], F32R, kind="ExternalInput").ap()
    mask_d = nc.dram_tensor("mask", [P, P], F32, kind="ExternalInput").ap()
    out_d = nc.dram_tensor("out", [T, DIN], F32, kind="ExternalOutput").ap()
    qt_d = nc.dram_tensor("qt_spill", [DLOC, T], F32R).ap()

    with nc.allow_low_precision(
        reason="fp32r matmuls: 4x PE throughput, ~2e-4 rel err"
    ), _SplitDrainTileContext(nc) as tc, ExitStack() as ctx:
        persist = ctx.enter_context(tc.tile_pool(name="persist", bufs=1))
        wpool = ctx.enter_context(tc.tile_pool(name="w", bufs=10))
        xpool = ctx.enter_context(tc.tile_pool(name="x", bufs=10))
        stage = ctx.enter_context(tc.tile_pool(name="stage", bufs=4))
        qrpool = ctx.enter_context(tc.tile_pool(name="qr", bufs=3))
        epool = ctx.enter_context(tc.tile_pool(name="e", bufs=6))
        rpool = ctx.enter_context(tc.tile_pool(name="r", bufs=2))
        rbpool = ctx.enter_context(tc.tile_pool(name="rb", bufs=2))
        # PSUM: the projection pool is scoped to the projection phase and
        # released before the attention pools allocate (8 banks total)
        ps_proj_cm = tc.tile_pool(name="ps_proj", bufs=2, space="PSUM")
        ps_proj = ps_proj_cm.__enter__()

        # ---- persistent SBUF buffers ----
        kt = [persist.tile([P, T], F32R, name=f"kt{i}", tag=f"kt{i}") for i in range(4)]
        ctxt = [
            persist.tile([P, T], F32R, name=f"ctxt{i}", tag=f"ctxt{i}") for i in range(4)
        ]
        va = persist.tile([P, NTC * NHL * VSLOT], F32R, name="va", tag="va")
        mask_sb = persist.tile([P, P], F32, name="mask_sb", tag="mask")
        ones_sb = persist.tile([1, DK], F32R, name="ones_sb", tag="ones")

        nc.sync.dma_start(out=mask_sb, in_=mask_d)
        # memset through f32 bitcast views: f32r memset fails an ISA check in
        # this walrus build, and 1.0 has identical bits in both formats
        nc.vector.memset(ones_sb.bitcast(F32), 1.0)
        va_view = va.rearrange("p (t h e) -> p t h e", h=NHL, e=VSLOT)
        va_view_f32 = va.bitcast(F32).rearrange("p (t h e) -> p t h e", h=NHL, e=VSLOT)
        nc.vector.memset(va_view_f32[:, :, :, DK : DK + 1], 1.0)

        # ---- V projection:  V[t, dv] = sum_k x_v^T[k, t] * Wv^T[k, dv] ----
        wv_sb = []
        for kc in range(KC):
            wt = wpool.tile([P, DLOC], F32R, name=f"wv{kc}", tag="w")
            nc.sync.dma_start(out=wt, in_=wv_d[kc * P : (kc + 1) * P, :])
            wv_sb.append(wt)
        for tg in range(NT):
            xcs = []
            for kc in range(KC):
                xc = xpool.tile([P, NQ], F32R, name=f"xv_{tg}_{kc}", tag="x")
                nc.sync.dma_start(
                    out=xc, in_=xv_d[kc * P : (kc + 1) * P, tg * NQ : (tg + 1) * NQ]
                )
                xcs.append(xc)
            for half in range(2):
                psums = [
                    ps_proj.tile([P, DLOC], F32, name=f"vps{half}_{i}", tag="pp")
                    for i in range(2)
                ]
                for kc in range(KC):
                    for i in range(2):
                        tsub = half * 2 + i
                        nc.tensor.matmul(
                            psums[i],
                            lhsT=xcs[kc][:, tsub * P : (tsub + 1) * P],
                            rhs=wv_sb[kc],
                            start=(kc == 0),
                            stop=(kc == KC - 1),
                        )
                for i in range(2):
                    tci = tg * 4 + half * 2 + i
                    nc.vector.tensor_copy(
                        out=va_view[:, tci, :, 0:DK],
                        in_=psums[i].rearrange("p (h e) -> p h e", e=DK),
                    )

        # ---- Q^T / K^T projections: out[m, t] = sum_k W^T[k, m] x^T[k, t] ----
        def qk_proj(w_dram, x_dram, sink, label):
            w_sb = []
            for kc in range(KC):
                wt = wpool.tile([P, DLOC], F32R, name=f"w{label}{kc}", tag="w")
                nc.sync.dma_start(out=wt, in_=w_dram[kc * P : (kc + 1) * P, :])
                w_sb.append(wt)
            for nt in range(NT):
                xcs = []
                for kc in range(KC):
                    xc = xpool.tile([P, NQ], F32R, name=f"x{label}_{nt}_{kc}", tag="x")
                    nc.sync.dma_start(
                        out=xc, in_=x_dram[kc * P : (kc + 1) * P, nt * NQ : (nt + 1) * NQ]
                    )
                    xcs.append(xc)
                for mh in range(2):
                    psums = [
                        ps_proj.tile([P, NQ], F32, name=f"{label}ps{mh}_{i}", tag="pp")
                        for i in range(2)
                    ]
                    for kc in range(KC):
                        for i in range(2):
                            mq = mh * 2 + i
                            nc.tensor.matmul(
                                psums[i],
                                lhsT=w_sb[kc][:, mq * P : (mq + 1) * P],
                                rhs=xcs[kc],
                                start=(kc == 0),
                                stop=(kc == KC - 1),
                            )
                    for i in range(2):
                        sink(mh * 2 + i, nt, psums[i])

        def q_sink(mq, nt, psum):
            st = stage.tile([P, NQ], F32R, name=f"qst{mq}_{nt}", tag="stage")
            nc.vector.tensor_copy(out=st, in_=psum)
            nc.sync.dma_start(
                out=qt_d[mq * P : (mq + 1) * P, nt * NQ : (nt + 1) * NQ], in_=st
            )

        def k_sink(mq, nt, psum):
            nc.vector.tensor_copy(
                out=kt[mq][:, nt * NQ : (nt + 1) * NQ], in_=psum
            )

        qk_proj(wq_d, xq_d, q_sink, "q")
        qk_proj(wk_d, xk_d, k_sink, "k")

        # release the projection PSUM pool; attention + out-proj pools below
        # reuse its banks (4 + 2 + 1 + 1 = 8 banks)
        ps_proj_cm.__exit__(None, None, None)
        ps_s = ctx.enter_context(tc.tile_pool(name="ps_s", bufs=4, space="PSUM"))
        ps_ctx = ctx.enter_context(tc.tile_pool(name="ps_ctx", bufs=2, space="PSUM"))
        ps_bc = ctx.enter_context(tc.tile_pool(name="ps_bc", bufs=1, space="PSUM"))
        ps_out = ctx.enter_context(tc.tile_pool(name="ps_out", bufs=1, space="PSUM"))

        wo_sb = []
        for kc4 in range(4):
            row = []
            for n in range(2):
                wt = wpool.tile([P, NQ], F32R, name=f"wo{kc4}_{n}", tag="w")
                nc.sync.dma_start(
                    out=wt,
                    in_=wo_d[kc4 * P : (kc4 + 1) * P, n * NQ : (n + 1) * NQ],
                )
                row.append(wt)
            wo_sb.append(row)

        # ---- attention ----
        # The two heads of each pair share one Q^T tile and are interleaved:
        # per j step the PE runs 4 matmuls (2 scores + 2 lagged ctx, ~each N
        # cycles) against 2 exp tiles on ACT, so ACT (the next-slowest
        # engine) stays off the critical path.
        def ctx_mm(hp, sub, et, off, j, jmax, cps):
            h = 2 * hp + sub
            nc.tensor.matmul(
                cps[sub] if j == 0 else cps[sub][:, off:NQ],
                lhsT=va_view[:, j, h, :],
                rhs=et if j == 0 else et[:, off:NQ],
                start=(j == 0),
                stop=(j == jmax - 1),
                skip_group_check=True,
            )

        for qi in range(NT):
            jmax = 4 * (qi + 1)
            for hp in range(NHL // 2):
                qt_t = qrpool.tile([P, NQ], F32R, name=f"qt{qi}_{hp}", tag="qr")
                nc.sync.dma_start(
                    out=qt_t,
                    in_=qt_d[hp * P : (hp + 1) * P, qi * NQ : (qi + 1) * NQ],
                )
                cps = [
                    ps_ctx.tile([VSLOT, NQ], F32, name=f"cps{qi}_{hp}_{s}", tag="ctx")
                    for s in range(2)
                ]
                pend = []  # [(sub, et, off, j)]
                for j in range(jmax):
                    delta = j * P - qi * NQ
                    off = max(0, delta)
                    cur = []
                    for sub in range(2):
                        h = 2 * hp + sub
                        krow = sub * DK
                        sps = ps_s.tile([P, NQ], F32, name=f"sps{qi}_{h}_{j}", tag="s")
                        nc.tensor.matmul(
                            sps[:, off:NQ],
                            lhsT=kt[hp][krow : krow + DK, j * P : (j + 1) * P],
                            rhs=qt_t[krow : krow + DK, off:NQ],
                            start=True,
                            stop=True,
                        )
                        cur.append((sub, sps))
                    for (sub, et, poff, pj) in pend:
                        ctx_mm(hp, sub, et, poff, pj, jmax, cps)
                    pend = []
                    for (sub, sps) in cur:
                        h = 2 * hp + sub
                        if delta >= 0:
                            nc.vector.tensor_add(
                                sps[:, off : off + P], sps[:, off : off + P], mask_sb
                            )
                        et = epool.tile([P, NQ], F32R, name=f"et{qi}_{h}_{j}", tag="e")
                        nc.scalar.activation(
                            out=et[:, off:NQ], in_=sps[:, off:NQ], func=EXP
                        )
                        pend.append((sub, et, off, j))
                for (sub, et, poff, pj) in pend:
                    ctx_mm(hp, sub, et, poff, pj, jmax, cps)

                # normalize: ctx[dv, q] / denom[q]
                for sub in range(2):
                    h = 2 * hp + sub
                    krow = sub * DK
                    rt = rpool.tile([1, NQ], F32R, name=f"rt{qi}_{h}", tag="recip")
                    nc.vector.reciprocal(rt, cps[sub][DK : DK + 1, :])
                    bc = ps_bc.tile([DK, NQ], F32, name=f"bc{qi}_{h}", tag="bc")
                    nc.tensor.matmul(bc, lhsT=ones_sb, rhs=rt, start=True, stop=True)
                    rb = rbpool.tile([DK, NQ], F32, name=f"rb{qi}_{h}", tag="rb")
                    nc.vector.tensor_copy(out=rb, in_=bc)
                    nc.vector.tensor_mul(
                        ctxt[hp][krow : krow + DK, qi * NQ : (qi + 1) * NQ],
                        cps[sub][0:DK, :],
                        rb,
                    )

            # ---- output projection for this q-block's t-chunks ----
            # out[t, n] = sum_dl ctx^T[dl, t] Wo^T[dl, n]; interleaving per
            # q-block overlaps its DVE/DMA epilogue with the next q-block
            for tsub in range(4):
                tci = qi * 4 + tsub
                for n in range(2):
                    ops = ps_out.tile([P, NQ], F32, name=f"ops{tci}_{n}", tag="out")
                    for kc4 in range(4):
                        nc.tensor.matmul(
                            ops,
                            lhsT=ctxt[kc4][:, tci * P : (tci + 1) * P],
                            rhs=wo_sb[kc4][n],
                            start=(kc4 == 0),
                            stop=(kc4 == 3),
                        )
                    st = stage.tile([P, NQ], F32, name=f"ost{tci}_{n}", tag="stage")
                    nc.vector.tensor_copy(out=st, in_=ops)
                    nc.sync.dma_start(
                        out=out_d[tci * P : (tci + 1) * P, n * NQ : (n + 1) * NQ],
                        in_=st,
                    )

    _split_excess_waits(nc)
    return nc


_NC_CACHE: bass.Bass | None = None


def _get_program() -> bass.Bass:
    global _NC_CACHE
    if _NC_CACHE is None:
        _NC_CACHE = _build_program()
    return _NC_CACHE


def _numpy_reference(q, k, v, Wq, Wk, Wv, Wo, bq, bk, bv, bo):
    """Exact fallback, used only if bq/bk/bv are nonzero (never the case for
    this problem's deterministic inputs)."""
    B, T_, D = q.shape
    H = 16
    dk = D // H

    def split(x):
        return x.reshape(B, T_, H, dk).transpose(0, 2, 1, 3)

    qh = split(q @ Wq.T + bq)
    kh = split(k @ Wk.T + bk)
    vh = split(v @ Wv.T + bv)
    scores = np.einsum("bhqd,bhkd->bhqk", qh, kh) / np.sqrt(np.float32(dk))
    causal = np.tril(np.ones((T_, T_), dtype=bool))
    scores = np.where(causal, scores, -np.inf).astype(np.float32)
    scores -= scores.max(axis=-1, keepdims=True)
    e = np.exp(scores)
    attn = e / e.sum(axis=-1, keepdims=True)
    ctx = np.einsum("bhqk,bhkd->bhqd", attn, vh)
    merged = ctx.transpose(0, 2, 1, 3).reshape(B, T_, D)
    return (merged @ Wo.T + bo).astype(np.float32)


def kernel(q, k, v, Wq, Wk, Wv, Wo, bq, bk, bv, bo):
    q, k, v = (np.asarray(a, np.float32) for a in (q, k, v))
    Wq, Wk, Wv, Wo = (np.asarray(a, np.float32) for a in (Wq, Wk, Wv, Wo))
    bq, bk, bv, bo = (np.asarray(a, np.float32) for a in (bq, bk, bv, bo))

    if np.any(bq) or np.any(bk) or np.any(bv):
        return _numpy_reference(q, k, v, Wq, Wk, Wv, Wo, bq, bk, bv, bo)

    B = q.shape[0]
    scale = np.float32(1.0 / np.sqrt(DK))
    wq_s = (Wq * scale).T  # fold score scale into Wq
    wk_s = Wk.T
    wv_s = Wv.T
    mask = np.where(
        np.arange(P)[:, None] <= np.arange(P)[None, :], 0.0, NEG
    ).astype(np.float32)

    in_maps = []
    for c in range(N_CORES):
        b, hh = divmod(c, 2)
        hs = slice(hh * DLOC, (hh + 1) * DLOC)
        in_maps.append(
            {
                "xq": np.ascontiguousarray(q[b].T),
                "xk": np.ascontiguousarray(k[b].T),
                "xv": np.ascontiguousarray(v[b].T),
                "wq": np.ascontiguousarray(wq_s[:, hs]),
                "wk": np.ascontiguousarray(wk_s[:, hs]),
                "wv": np.ascontiguousarray(wv_s[:, hs]),
                "wo": np.ascontiguousarray(Wo[:, hs].T),
                "mask": mask,
            }
        )

    nc = _get_program()
    res = bass_utils.run_bass_kernel_spmd(nc, in_maps, core_ids=list(range(N_CORES)))

    out = np.empty((B, T, DIN), np.float32)
    for b in range(B):
        out[b] = res.results[2 * b]["out"] + res.results[2 * b + 1]["out"]
    out += bo
    return out
